# revision 1
# baseline (speedup 1.0000x reference)
"""Pairwise squared-Euclidean distance kernel for TRN2 (8 NeuronCores).

Problem: matrix_1 [8, 2048, 256] fp32 -> out [8, 2048, 2048] fp32 with
  out[b,i,j] = max(||x_i||^2 + ||x_j||^2 - 2 x_i.x_j, 0)

Sharding: data-parallel over batch; core b handles matrix_1[b] entirely.

Per-core plan (X = [2048, 256]):
  1. DMA X in as 16 [128, 256] tiles.
  2. PE-transpose each tile's two 128-wide k-chunks into PSUM strips,
     copy to SBUF -> XT0/XT1 [128, 2048] (X^T, k on partitions).
     XT serves as BOTH matmul operands (lhsT and rhs) since the Gram
     matrix is X @ X^T.
  3. Row norms NI [128, 16] via fused DVE tensor_tensor_reduce
     (square + free-axis sum per tile).
  4. NJ [128, 2048] = col-norms replicated over partitions via
     ones[128,128].T @ (XT*XT)  (partition-axis reduction on PE).
  5. Main loop over 16 row blocks i:
       psum[128,2048] (4 banks) <- 8 matmuls (4 col blocks x 2 k-chunks)
       s = Identity(-2*psum + NI[:,i])        (ACT, bias per-partition)
       m = max(s, -NJ); d = m + NJ            (DVE; == relu(s + NJ))
       DMA d -> out rows (1 MiB contiguous per block)
"""

import os

import numpy as np

import concourse.bass as bass
import concourse.mybir as mybir
from concourse import bacc, masks, tile
from concourse.bass_utils import run_bass_kernel_spmd

B, S, R = 8, 2048, 256
P = 128            # SBUF partitions
NT = S // P        # 16 row blocks
NBW = 512          # matmul moving-dim block = one fp32 PSUM bank
NB = S // NBW      # 4 col blocks
KH = R // P        # 2 contraction chunks

F32 = mybir.dt.float32


def _mm_dtype():
    # float32r: single-pass reduced-precision fp32 multiply, 4x faster on PE.
    name = os.environ.get("KNN_MM_DTYPE", "f32r")
    return F32 if name == "f32" else mybir.dt.float32r


_ldw_patched = False


def _maybe_enable_ldw_opt():
    """Rewrite walrus's hardcoded --enable-ldw-opt=false when requested."""
    global _ldw_patched
    if _ldw_patched or os.environ.get("KNN_LDW_OPT", "0") != "1":
        return
    from concourse import bass_utils as bu

    orig = bu.run_command

    def patched(argv, **kw):
        argv = ["--enable-ldw-opt=true" if a == "--enable-ldw-opt=false" else a
                for a in argv]
        return orig(argv, **kw)

    bu.run_command = patched
    _ldw_patched = True


def build_nc(mm_dt=None):
    if mm_dt is None:
        mm_dt = _mm_dtype()
    _maybe_enable_ldw_opt()
    # Bacc (not plain Bass): its compile() runs move_matmul_waits_to_ldweights
    # + generate_event_semaphores, without which walrus rejects matmuls that
    # accumulated >1 semaphore wait ("Too many sync wait commands").
    nc = bacc.Bacc()
    x = nc.declare_dram_parameter("x", [S, R], F32, isOutput=False)
    out = nc.declare_dram_parameter("out", [S, S], F32, isOutput=True)

    with tile.TileContext(nc) as tc:
        with (
            tc.tile_pool(name="const", bufs=1) as cpool,
            tc.tile_pool(name="xin", bufs=6) as xin_pool,
            tc.tile_pool(name="xt", bufs=1) as xt_pool,
            tc.tile_pool(name="nrm", bufs=1) as nrm_pool,
            tc.tile_pool(name="scr", bufs=3) as scr_pool,
            tc.tile_pool(name="stile", bufs=3) as s_pool,
            tc.tile_pool(name="obuf", bufs=4) as o_pool,
            tc.tile_pool(name="psum", bufs=2, space="PSUM") as psum_pool,
        ):
            ident = cpool.tile([P, P], F32)
            masks.make_identity(nc, ident[:])
            if os.environ.get("KNN_LDW_OPT", "0") == "1":
                # NEFF cache keys on BIR only, not walrus flags — perturb it
                cachebust = cpool.tile([P, 1], F32)
                nc.gpsimd.memset(cachebust[:], 2.0)
            # Matmul operand tiles carry the matmul dtype: the BIR verifier
            # requires f32r matmul inputs to be *produced* rounded-to-f32r
            # (bitcasting plain f32 APs at the matmul is rejected).
            if mm_dt is F32:
                ones = cpool.tile([P, P], F32)
                nc.gpsimd.memset(ones[:], 1.0)
            else:
                # memset can't emit f32r; round through a DVE copy
                onesf = cpool.tile([P, P], F32)
                nc.gpsimd.memset(onesf[:], 1.0)
                ones = cpool.tile([P, P], mm_dt)
                nc.vector.tensor_copy(ones[:], onesf[:])

            XT0 = xt_pool.tile([P, S], mm_dt)
            XT1 = xt_pool.tile([P, S], mm_dt)
            XTs = [XT0, XT1]
            XSQ0 = xt_pool.tile([P, S], mm_dt)
            XSQ1 = xt_pool.tile([P, S], mm_dt)
            NI = nrm_pool.tile([P, NT], F32)
            NJ = nrm_pool.tile([P, S], F32)

            # --- prologue: load, transpose, row norms ---
            # f32r transposes run at 1.5 cyc/row vs 2.0 for fp32; the values
            # get f32r-rounded at the XT cast anyway.
            # (default off: the verifier rejects bitcast-f32r transpose inputs
            # whose producer is a DMA — only rounding ops may produce f32r)
            tr_f32r = (mm_dt is not F32) and os.environ.get("KNN_TR_F32R", "0") == "1"
            tr_dt = mm_dt if tr_f32r else F32
            if tr_f32r:
                identr = cpool.tile([P, P], mm_dt)
                nc.vector.tensor_copy(identr[:], ident[:])
            else:
                identr = ident
            strip0 = psum_pool.tile([P, S], tr_dt, tag="psrow")
            strip1 = psum_pool.tile([P, S], tr_dt, tag="psrow")
            for t in range(NT):
                xin = xin_pool.tile([P, R], F32, tag="xin")
                nc.sync.dma_start(xin[:], x[t * P:(t + 1) * P, :])
                xtr = xin[:].bitcast(tr_dt)
                nc.tensor.transpose(strip0[:, t * P:(t + 1) * P], xtr[:, 0:P], identr[:])
                nc.tensor.transpose(strip1[:, t * P:(t + 1) * P], xtr[:, P:R], identr[:])
                # row norms on ACT: Square + free-axis accumulate
                # (tensor_tensor_reduce on DVE crashes the device — NRT exec
                # error; ACT accum keeps DVE free for the epilogue.)
                scr = scr_pool.tile([P, R], F32, tag="scr")
                nc.scalar.activation(
                    scr[:], xin[:], mybir.ActivationFunctionType.Square,
                    accum_out=NI[:, t:t + 1],
                )
            nc.vector.tensor_copy(XT0[:], strip0[:])
            nc.vector.tensor_copy(XT1[:], strip1[:])

            # --- NJ: column norms replicated across partitions ---
            nc.vector.tensor_mul(XSQ0[:], XT0[:], XT0[:])
            nc.vector.tensor_mul(XSQ1[:], XT1[:], XT1[:])
            njp = psum_pool.tile([P, S], F32, tag="psrow")
            for j in range(NB):
                jsl = slice(j * NBW, (j + 1) * NBW)
                nc.tensor.matmul(njp[:, jsl], ones[:], XSQ0[:, jsl], start=True, stop=False)
                nc.tensor.matmul(njp[:, jsl], ones[:], XSQ1[:, jsl], start=False, stop=True)
            nc.vector.tensor_copy(NJ[:], njp[:])

            # --- main loop over row blocks ---
            for i in range(NT):
                isl = slice(i * P, (i + 1) * P)
                ps = psum_pool.tile([P, S], F32, tag="psrow")
                for k in range(KH):
                    for j in range(NB):
                        jsl = slice(j * NBW, (j + 1) * NBW)
                        nc.tensor.matmul(
                            ps[:, jsl],
                            XTs[k][:, isl],
                            XTs[k][:, jsl],
                            start=(k == 0),
                            stop=(k == KH - 1),
                        )
                # Epilogue in two column halves (pipelines stt->relu->DMA).
                # (GpSimd relu was tried and is ~10x slower than ACT — 279us
                # total vs 85us; Pool-engine fp32 tensor_scalar is not viable.)
                s = s_pool.tile([P, S], F32, tag="s")
                d = o_pool.tile([P, S], F32, tag="d")
                for h in range(2):
                    hsl = slice(h * (S // 2), (h + 1) * (S // 2))
                    nc.vector.scalar_tensor_tensor(
                        out=s[:, hsl], in0=ps[:, hsl], scalar=-2.0, in1=NJ[:, hsl],
                        op0=mybir.AluOpType.mult, op1=mybir.AluOpType.add,
                    )
                    nc.scalar.activation(
                        d[:, hsl], s[:, hsl], mybir.ActivationFunctionType.Relu,
                        bias=NI[:, i:i + 1], scale=1.0,
                    )
                nc.sync.dma_start(out[isl, :], d[:])

    return nc


_cached_nc = None


def run(matrix_1, trace=False, tmpdir=None, mm_dt=None, **spmd_kwargs):
    """Run the SPMD kernel on 8 cores; returns (out [8,S,S], BassKernelResults)."""
    global _cached_nc
    if _cached_nc is None or mm_dt is not None:
        nc = build_nc(mm_dt)
        if mm_dt is None:
            _cached_nc = nc
    else:
        nc = _cached_nc
    # The axon/PJRT path serializes nc as-is; Bacc's compile() (reg alloc,
    # matmul wait splitting) only runs inside finalize(), so do it here.
    if not nc.is_finalized():
        nc.finalize()
    matrix_1 = np.ascontiguousarray(np.asarray(matrix_1, dtype=np.float32))
    assert matrix_1.shape == (B, S, R)
    in_maps = [{"x": matrix_1[b]} for b in range(B)]
    try:
        res = run_bass_kernel_spmd(
            nc, in_maps, list(range(B)), tmpdir=tmpdir, trace=trace, **spmd_kwargs
        )
    except Exception:
        # transient device wedges (NRT_EXEC_UNIT_UNRECOVERABLE) clear on retry
        res = run_bass_kernel_spmd(
            nc, in_maps, list(range(B)), tmpdir=tmpdir, trace=trace, **spmd_kwargs
        )
    out = np.stack([res.results[b]["out"] for b in range(B)], axis=0)
    return out, res


def kernel(matrix_1):
    out, _ = run(matrix_1)
    return out

